# revision 2
# baseline (speedup 1.0000x reference)
"""Trainium2 Bass kernel for nn_AttentionBlock (B=8, C=256, H=W=32, 8 heads, dk=64).

Sharding: data-parallel over batch B across the 8 NeuronCores (one batch
element per core, weights replicated, no collectives).

v2 changes over the original (126.6us -> ~120.7us):
  - Startup: contiguous f32->fp16 SWDGE loads of x / w_qkv (big descriptors,
    the q/k column reorder done by APs on the matmul stationary operands,
    per-head M=64 col-tiles since stationary APs are single-free-dim only).
    The non-critical halves (w_qkv cols for pairs 2-3, w_out, x f32) are
    gated behind the critical loads via a WAW-dependency scribble, because
    the DMA engines round-robin across queued transfers and everything
    co-finishes otherwise.
  - Softmax exp (64 tiles of [128,1024]; the original scalar-engine
    bottleneck at ~80us busy) split: 44 tiles on ScalarE activation-exp
    (fused accum), 20 tiles via a Schraudolph exp2 bit-trick on DVE
    (P_i16 = round(A*T + B) in one tensor_scalar pass; the fp16 BITCAST of
    that tile IS exp(T*SCALE) to ~2% rms and feeds AV directly; row sums
    via a separate fp16 tensor_reduce).  GpSimd cannot touch PSUM or run
    accum ops, so it instead owns the per-pair v-row 1/s scaling (batched
    tensor_tensor with a stride-0 broadcast of the reciprocal columns).
  - Pipeline deepened: v-scale normalization at J-2, AV consumption at J-3,
    consumers emitted ahead of producers inside each step; the res copy is
    deferred into the next phase's fill queue; res_ps has its own psum tag
    and proj/work psums share the T ring.
  - Matmul structure as before (fp16; AV + projection col-tile pairs run
    concurrently on the PE).  Remaining limiter: the PE only reaches its
    full 2.4GHz p-state under ~3us of gapless execution, which the
    exp-dependent gaps prevent, so T matmuls mostly stream at 1.2GHz.
"""

import os
import sys

import numpy as np

for _p in ("/opt/trn_rl_repo",):
    if os.path.isdir(_p) and _p not in sys.path:
        sys.path.insert(0, _p)

import concourse.bass as bass
import concourse.mybir as mybir
import concourse.tile as tile
from concourse import bacc
from concourse.bass_utils import run_bass_kernel_spmd

F32 = mybir.dt.float32
FP16 = mybir.dt.float16
I16 = mybir.dt.int16
AF = mybir.ActivationFunctionType
ALU = mybir.AluOpType

N_HEADS = 8
DK = 64
C = 256
S = 1024
INNER = N_HEADS * DK  # 512
SCALE = DK ** -0.5
B = 8

# Schraudolph fp16 exp2 constants (SCALE folded into the multiplier):
# exp(T*SCALE) ~= bitcast_fp16(int16(round(A_EXP*T + B_EXP)))
A_EXP = 1024.0 * 1.4426950408889634 * SCALE
B_EXP = 15.0 * 1024.0 - 59.3
GP_VSCALE = True

# exp engine assignment: (J, hi) -> 'sc' (ScalarE) | 'dve' | 'gp'
# (same per pair-phase p).  ~40 sc / 8 dve / 16 gp.
ASSIGN = {}
for _J in range(8):
    for _hi in range(2):
        ASSIGN[(_J, _hi)] = "sc"
for _J in (1, 3, 5, 7):
    ASSIGN[(_J, 0)] = "dve"
ASSIGN[(3, 1)] = "dve"


def _body(nc, tc, ctx, x_d, wqkv_d, bqkv_d, wout_d, bout_d, y_d):
    sb = ctx.enter_context(tc.tile_pool(name="sb", bufs=1))
    sbP = ctx.enter_context(tc.tile_pool(name="sbP", bufs=1))
    ps = ctx.enter_context(tc.tile_pool(name="ps", bufs=1, space="PSUM"))

    # ---- persistent SBUF tensors ----
    x_sb = sb.tile([128, 2, S], F32)        # x_b as 2 channel tiles (residual)
    x16 = sb.tile([128, 2, S], FP16)
    w16 = sb.tile([128, 2, 8, 3, 64], FP16)  # w_qkv fp16 [c-tile, ct, h, qkv, d]
    wo16 = sb.tile([128, 4, C], FP16)
    qq_sb = sb.tile([128, 4, S], FP16)      # q^T head-pair tiles
    kk_sb = sb.tile([128, 4, S], FP16)      # k^T head-pair tiles
    v_sb = sb.tile([128, 8, INNER], FP16)   # v token tiles, head-major cols
    res_sb = sb.tile([128, 4, S], FP16)     # res^T feature tiles
    out_sb = sb.tile([128, 2, S], F32)
    bq_sb = sb.tile([128, 4], F32)          # per-pair q bias columns
    bk_sb = sb.tile([128, 4], F32)
    bv_row = sb.tile([1, INNER], FP16)      # v bias as a single row
    ones_row = sb.tile([1, 128], FP16)
    bo_sb = sb.tile([128, 2], F32)
    s_sb = sb.tile([128, 64, 1], F32)       # softmax denominators
    rs_sb = sb.tile([128, 64, 1], F32)
    warm = sb.tile([1, 2], FP16)            # act-table warmup scratch

    # ---- input DMAs ----
    # Contiguous converting SWDGE loads (f32 DRAM -> fp16 SBUF, one descriptor
    # per partition row).  Column reorder for q/k/v happens via APs at matmul
    # emission.  Order: first the operands gating the pair-0 projections.
    gate = sb.tile([1, 8], FP16)
    for ct in range(2):
        nc.gpsimd.dma_start(
            out=w16[:, ct, 0:4, :, :].rearrange("p h t d -> p (h t d)"),
            in_=wqkv_d[128 * ct:128 * (ct + 1), 0:768])
        nc.gpsimd.dma_start(out=x16[:, ct, :],
                            in_=x_d[128 * ct:128 * (ct + 1), :])
    # gate the bulk of the weights behind the critical pair-0/1 slices: the
    # DMA engines round-robin across queued transfers, so issuing everything
    # at once makes the critical loads finish last.  A scribble into the
    # back half (gated on x16-ct0) forces a WAW dependency on the back DMAs
    # so they are not even enqueued until the critical transfers complete.
    nc.gpsimd.tensor_copy(out=w16[0:1, 0, 4, 0, 0:8], in_=x16[0:1, 0, 0:8])
    nc.gpsimd.tensor_copy(out=w16[0:1, 1, 4, 0, 0:8], in_=x16[0:1, 0, 0:8])
    for ct in range(2):
        nc.gpsimd.dma_start(
            out=w16[:, ct, 4:8, :, :].rearrange("p h t d -> p (h t d)"),
            in_=wqkv_d[128 * ct:128 * (ct + 1), 768:1536])
    bv_src = bass.AP(tensor=bqkv_d.tensor, offset=128, ap=[[192, 8], [1, 64]])
    nc.gpsimd.dma_start(
        out=bv_row[:, :].rearrange("p (h d) -> p h d", h=8, d=64), in_=bv_src)

    def load_wo(ft):
        nc.gpsimd.dma_start(out=wo16[:, ft, :],
                            in_=wout_d[128 * ft:128 * (ft + 1), :])

    def load_xf32(ct):
        # f32 residual copy of x (only needed at the out-proj fills)
        nc.sync.dma_start(out=x_sb[:, ct, :], in_=x_d[128 * ct:128 * (ct + 1), :])
    # q/k bias gathers -> [128 (=2 heads x 64 d), 4 pairs]
    for off, btile in ((0, bq_sb), (64, bk_sb)):
        for hh in range(2):
            src = bass.AP(tensor=bqkv_d.tensor, offset=off + 192 * hh,
                          ap=[[1, 64], [384, 4]])
            nc.scalar.dma_start(out=btile[64 * hh:64 * (hh + 1), :], in_=src)
    bo_src = bass.AP(tensor=bout_d.tensor, offset=0, ap=[[1, 128], [128, 2]])
    nc.scalar.dma_start(out=bo_sb[:, :], in_=bo_src)
    nc.vector.memset(ones_row[:, :], 1.0)
    # preload the exp activation table while DMAs stream
    nc.scalar.activation(warm[:, :], ones_row[:, 0:2], AF.Exp)
    wv16 = sb.tile([128, 2, INNER], FP16)

    # ---- deferred PE work units (emitted into the attention pipeline) ----
    def emit_qk(p, t_idx, ih):
        dst, btile = ((qq_sb, bq_sb), (kk_sb, bk_sb))[t_idx]
        g = ps.tile([128, 512], F32, tag="T", bufs=3,
                    name=f"qk_ps_{p}_{t_idx}_{ih}")
        for ct in range(2):
            for hh in range(2):
                # stationary APs are single-free-dim only: per-head M=64
                # col-tiles (the hh pair runs concurrently on the PE)
                nc.tensor.matmul(
                    g[64 * hh:64 * hh + 64, :],
                    lhsT=w16[:, ct, 2 * p + hh, t_idx, :],
                    rhs=x16[:, ct, 512 * ih:512 * (ih + 1)],
                    start=(ct == 0), stop=(ct == 1),
                    skip_group_check=True,
                )
        if (p + t_idx) % 2 == 0:
            nc.vector.tensor_scalar_add(
                out=dst[:, p, 512 * ih:512 * (ih + 1)], in0=g,
                scalar1=btile[:, p:p + 1],
            )
        else:
            nc.scalar.activation(
                dst[:, p, 512 * ih:512 * (ih + 1)], g, AF.Identity,
                bias=btile[:, p:p + 1])

    def emit_v(tt):
        g = ps.tile([128, 512], F32, tag="T", bufs=3, name=f"v_ps_{tt}")
        for ct in range(2):
            nc.tensor.matmul(
                g[:, :],
                lhsT=x16[:, ct, 128 * tt:128 * (tt + 1)],
                rhs=wv16[:, ct, :],
                start=(ct == 0), stop=False,
            )
        # bias via rank-1 matmul: out[token, f] += 1 * b_v[f]
        nc.tensor.matmul(
            g[:, :], lhsT=ones_row[:, :], rhs=bv_row[:, :],
            start=False, stop=True,
        )
        nc.vector.tensor_copy(out=v_sb[:, tt, :], in_=g)

    def emit_out01(m, ih):
        g = ps.tile([128, 512], F32, tag="T", bufs=3, name=f"o01_{m}_{ih}")
        for ft in range(2):
            nc.tensor.matmul(
                g[:, :],
                lhsT=wo16[:, ft, 128 * m:128 * (m + 1)],
                rhs=res_sb[:, ft, 512 * ih:512 * (ih + 1)],
                start=(ft == 0), stop=(ft == 1),
            )
        # fold the residual in here; the tail adds the ft2/3 half + bias
        nc.vector.tensor_tensor(
            out=out_sb[:, m, 512 * ih:512 * (ih + 1)], in0=g,
            in1=x_sb[:, m, 512 * ih:512 * (ih + 1)], op=ALU.add)

    # qq/kk for pair 0 gate the whole pipeline: emit first
    for t_idx in range(2):
        emit_qk(0, t_idx, 0)
        emit_qk(0, t_idx, 1)
    # contiguous copy of the v weight columns (matmul moving operands must
    # have a single free dim); emitted after pair-0's bias copies so the
    # vector queue reaches those before waiting on the gated weight half
    for ct in range(2):
        nc.vector.tensor_copy(out=wv16[:, ct, :], in_=w16[:, ct, :, 2, :])

    # per-phase fill queues, consumed one chunk per pipeline step (leftovers
    # drain at the phase end)
    fills = {
        0: [lambda tt=tt: emit_v(tt) for tt in range(8)]
           + [lambda ih=ih, t=t: emit_qk(1, t, ih)
              for ih in range(2) for t in range(2)],
        1: [lambda ct=ct: load_xf32(ct) for ct in range(2)]
           + [lambda ft=ft: load_wo(ft) for ft in range(4)]
           + [lambda ih=ih, t=t: emit_qk(2, t, ih)
              for ih in range(2) for t in range(2)],
        2: [lambda ih=ih, t=t: emit_qk(3, t, ih)
            for ih in range(2) for t in range(2)]
           + [lambda m=m: emit_out01(m, 0) for m in range(2)],
        3: [lambda m=m: emit_out01(m, 1) for m in range(2)],
    }

    # ---- exp dispatch: ScalarE activation or Schraudolph on DVE/GpSimd ----
    def emit_exp(eng, Pt, Tp, c):
        if eng == "sc":
            nc.scalar.activation(
                Pt[:, :], Tp, AF.Exp, scale=SCALE,
                accum_out=s_sb[:, c, :],
            )
            return
        # Schraudolph on DVE (gpsimd cannot access PSUM nor run accum ops):
        # pass1 writes P directly as i16 (its fp16 bitcast IS exp(T*SCALE));
        # the row sums come from a separate fp16 reduce.
        nc.vector.tensor_scalar(out=Pt, in0=Tp, scalar1=A_EXP,
                                scalar2=B_EXP, op0=ALU.mult, op1=ALU.add)
        nc.vector.tensor_reduce(out=s_sb[:, c, :], in_=Pt.bitcast(FP16),
                                axis=mybir.AxisListType.X, op=ALU.add)

    # ---- attention: software-pipelined per key-tile J ----
    P_tiles = {}
    LAG = 3
    for p in range(4):
        # both heads of the pair accumulate into one psum tensor: head hi=0
        # in partitions 0-63, hi=1 in 64-127 (col tiling)
        res_ps = ps.tile([128, S], F32, tag="res", bufs=1, name=f"res_ps_{p}")
        fill = fills[p]
        for step in range(8 + LAG):
            # normalization for step-2's tiles first (consumers lead the
            # queues): one reciprocal for the pair, then fold 1/s into the
            # v rows of that key tile on gpsimd
            Jn = step - 2
            if 0 <= Jn < 8:
                c0 = 16 * p + 2 * Jn
                nc.vector.reciprocal(rs_sb[:, c0:c0 + 2, :],
                                     s_sb[:, c0:c0 + 2, :])
                vsl = v_sb[:, Jn, 128 * p:128 * p + 128]
                if GP_VSCALE:
                    v3 = vsl.rearrange("p (h d) -> p h d", h=2, d=64)
                    nc.gpsimd.tensor_tensor(
                        out=v3, in0=v3,
                        in1=rs_sb[:, c0:c0 + 2, :].to_broadcast((128, 2, 64)),
                        op=ALU.mult)
                else:
                    for hi in range(2):
                        vs = v_sb[:, Jn, 64 * (2 * p + hi):64 * (2 * p + hi) + 64]
                        nc.vector.tensor_scalar_mul(
                            out=vs, in0=vs,
                            scalar1=rs_sb[:, c0 + hi, :])
            Jav = step - LAG
            if Jav >= 0:
                for ih in range(2):
                    for hi in range(2):
                        h = 2 * p + hi
                        # sim's zero-region group check drops the partition
                        # base and false-positives on this col-tiled pattern
                        nc.tensor.matmul(
                            res_ps[64 * hi:64 * hi + 64, 512 * ih:512 * (ih + 1)],
                            lhsT=v_sb[:, Jav, 64 * h:64 * h + 64],
                            rhs=P_tiles[(h, Jav)][:, 512 * ih:512 * (ih + 1)],
                            start=(Jav == 0), stop=(Jav == 7),
                            skip_group_check=True,
                        )
            J = step
            if J < 8:
                for hi in range(2):
                    h = 2 * p + hi
                    Tp = ps.tile([128, S], F32, tag="T", bufs=3, name=f"T_{h}_{J}")
                    for ih in range(2):
                        # T[j, i] = sum_d k[j, d] q[i, d]
                        nc.tensor.matmul(
                            Tp[:, 512 * ih:512 * (ih + 1)],
                            lhsT=kk_sb[64 * hi:64 * hi + 64, p,
                                       128 * J:128 * (J + 1)],
                            rhs=qq_sb[64 * hi:64 * hi + 64, p,
                                      512 * ih:512 * (ih + 1)],
                            start=True, stop=True,
                        )
                    eng = ASSIGN[(J, hi)]
                    c = 16 * p + 2 * J + hi
                    Pt = sbP.tile([128, S], FP16 if eng == "sc" else I16,
                                  tag="P", bufs=16, name=f"P_{h}_{J}")
                    emit_exp(eng, Pt, Tp, c)
                    P_tiles[(h, J)] = (Pt[:, :] if eng == "sc"
                                       else Pt[:, :].bitcast(FP16))
            if fill:
                fill.pop(0)()
        while fill:
            fill.pop(0)()
        # the res copy is deferred into the next phase's fill queue so the
        # scalar engine never stalls on the phase drain
        def res_copy(p=p, res_ps=res_ps):
            nc.scalar.copy(out=res_sb[:, p, :], in_=res_ps)
        if p < 3:
            fills[p + 1].insert(0, res_copy)
        else:
            res_copy()
        for J in range(8):
            for hi in range(2):
                del P_tiles[(2 * p + hi, J)]

    # ---- output projection tail: ft 2-3 half + bias, then store ----
    for m in range(2):
        for ih in range(2):
            g = ps.tile([128, 512], F32, tag="T", bufs=3, name=f"o23_{m}_{ih}")
            for ft in (2, 3):
                nc.tensor.matmul(
                    g[:, :],
                    lhsT=wo16[:, ft, 128 * m:128 * (m + 1)],
                    rhs=res_sb[:, ft, 512 * ih:512 * (ih + 1)],
                    start=(ft == 2), stop=(ft == 3),
                )
            nc.vector.scalar_tensor_tensor(
                out=out_sb[:, m, 512 * ih:512 * (ih + 1)],
                in0=g, scalar=bo_sb[:, m:m + 1],
                in1=out_sb[:, m, 512 * ih:512 * (ih + 1)],
                op0=ALU.add, op1=ALU.add,
            )
            nc.sync.dma_start(
                out=y_d[128 * m:128 * (m + 1), 512 * ih:512 * (ih + 1)],
                in_=out_sb[:, m, 512 * ih:512 * (ih + 1)])


_NC_CACHE = None


def _build_nc():
    global _NC_CACHE
    if _NC_CACHE is not None:
        return _NC_CACHE
    nc = bacc.Bacc("TRN2", target_bir_lowering=False)
    x_d = nc.dram_tensor("x", [C, S], F32, kind="ExternalInput")
    wqkv_d = nc.dram_tensor("w_qkv", [C, 3 * INNER], F32, kind="ExternalInput")
    bqkv_d = nc.dram_tensor("b_qkv", [3 * INNER], F32, kind="ExternalInput")
    wout_d = nc.dram_tensor("w_out", [INNER, C], F32, kind="ExternalInput")
    bout_d = nc.dram_tensor("b_out", [C], F32, kind="ExternalInput")
    y_d = nc.dram_tensor("y", [C, S], F32, kind="ExternalOutput")
    from contextlib import ExitStack
    with tile.TileContext(nc) as tc, ExitStack() as ctx:
        _body(nc, tc, ctx, x_d.ap(), wqkv_d.ap(), bqkv_d.ap(), wout_d.ap(),
              bout_d.ap(), y_d.ap())
    nc.compile()
    _NC_CACHE = nc
    return nc


def kernel(x, w_qkv, b_qkv, w_out, b_out, _trace=False, _tmpdir=None):
    x = np.ascontiguousarray(np.asarray(x, dtype=np.float32))
    w_qkv = np.ascontiguousarray(np.asarray(w_qkv, dtype=np.float32))
    b_qkv = np.ascontiguousarray(np.asarray(b_qkv, dtype=np.float32))
    w_out = np.ascontiguousarray(np.asarray(w_out, dtype=np.float32))
    b_out = np.ascontiguousarray(np.asarray(b_out, dtype=np.float32))

    nc = _build_nc()
    in_maps = [
        {
            "x": x[b].reshape(C, S),
            "w_qkv": w_qkv,
            "b_qkv": b_qkv,
            "w_out": w_out,
            "b_out": b_out,
        }
        for b in range(B)
    ]
    kw = {}
    if _trace:
        kw = {"trace": True, "tmpdir": _tmpdir}
    r = run_bass_kernel_spmd(nc, in_maps, core_ids=list(range(B)), **kw)
    y = np.stack([m["y"] for m in r.results], axis=0).reshape(B, C, 32, 32)
    if _trace:
        kernel.last_results = r
    return y
